# revision 1
# baseline (speedup 1.0000x reference)
"""Trainium2 Bass kernel for nn_DirectionalAttn (directional sparse attention).

Computation (per batch item b):
    rep = elu(x @ W_rep + b_rep)                       # [S, D]
    dep = rep @ W_dep;  head = rep @ W_head            # [S, D]
    T[i,j,d]  = tanh((dep[j,d] + head[i,d] + b_attn[d]) / 5)
    E[i,j,d]  = exp(5 * T[i,j,d]) * (j < i)
    attn[i,d] = sum_j E*rep[j,d] / sum_j E
    gate = sigmoid(attn @ W_fa + rep @ W_fr + b_f)
    out  = gate*rep + (1-gate)*attn

Sharding: data-parallel over batch, 2 items per core across 8 cores; the
small DxD weights are replicated.  rep_mask is all-ones per the problem
spec and is not consumed.

Per-core layout for the S^2 x D core: d-halves (128 partitions) x (i,j)
in the free dimension, block-prefix tiled (i-blocks of BLK=16, j spanning
[0, BLK*(bi+1))).  The broadcast sum U[d,(i,j)] = dep[j,d] + head[i,d] is
materialized by the tensor engine as four accumulating compensated-
float32r matmuls (dep/head each split into hi + lo f32r parts, so the
sum is fp32-exact at 6 PE cycles/col vs 8 for fp32) whose moving
operands are broadcast access patterns over one SBUF identity matrix
(selector columns).  tanh runs on the scalar engine with
the (+b_attn)/SCALE fold in its per-partition affine; exp follows
immediately (same engine, same table set); the causal mask inside
diagonal blocks zeroes E via gpsimd affine_select (fill=0.0); the vector
engine does the segmented reductions while gpsimd takes most of the
E*rep multiply.  bi-rounds are emitted stage-major across the four
(b, h) units to keep every engine's strict-FIFO queue dense.
"""

import numpy as np

import concourse.bacc as bacc
import concourse.bass as bass
import concourse.mybir as mybir
import concourse.tile as tile
from concourse.bass_utils import run_bass_kernel_spmd
from concourse.masks import make_identity

B, S, D = 16, 128, 256
NCORES = 8
BC = B // NCORES          # batch items per core
NH = D // 128             # d-halves
SCALE = 5.0
DEN_EPS = 1e-30           # guards 0/0 for row i=0 (fully masked)
FP = mybir.dt.float32
AF = mybir.ActivationFunctionType
ALU = mybir.AluOpType

# i-block bi has j-span L = BLK*(bi+1); PSUM chunks are [128, 1536] (3
# banks), one piece per bank so each matmul stays within a bank.
# n_i per piece: largest divisor of BLK with n_i*L <= 512.
BLK = 16
NBI = 128 // BLK
_BLOCK_PLAN = []  # bi -> list of chunks; chunk = list of (i_start, n_i)
for _bi in range(NBI):
    _L = BLK * (_bi + 1)
    _ni = BLK
    while _ni * _L > 512:
        _ni //= 2
    _pieces = [(_i, _ni) for _i in range(0, BLK, _ni)]
    _BLOCK_PLAN.append([_pieces[k:k + 2] for k in range(0, len(_pieces), 2)])


def _bcast(ap, free_pattern, extra_offset=0):
    """AP over `ap`'s tensor keeping its partition dim, with a custom free
    pattern (supports step-0 broadcast entries).  Offsets are in elements."""
    return bass.AP(
        tensor=ap.tensor,
        offset=ap.offset + extra_offset,
        ap=[list(ap.ap[0])] + [list(p) for p in free_pattern],
    )


USE_F32R = False


def _rr(ap):
    """Unconditional float32r view (phase-B compensated hi+lo matmuls)."""
    return ap.bitcast(mybir.dt.float32r)


def _r(ap):
    """View an fp32 AP as float32r (PE: 1.5 cyc/col vs 4 for fp32; TF32-like
    ~2^-12 operand rounding -> ~3e-4 output rel err).  USE_F32R=False keeps
    full fp32 (~1.8e-6) at ~15-20% more wall time."""
    if not USE_F32R:
        return ap
    return ap.bitcast(mybir.dt.float32r)


def build_program(reps=1):
    """Build + compile the per-core program. reps>1 wraps the computation
    in a For_i loop (used for wall-clock slope timing)."""
    nc = bacc.Bacc("TRN2", target_bir_lowering=False, debug=False,
                   num_devices=NCORES)

    x_d = nc.dram_tensor("x", [BC, S, D], FP, kind="ExternalInput")
    w_rep_d = nc.dram_tensor("w_rep", [D, D], FP, kind="ExternalInput")
    b_rep_d = nc.dram_tensor("b_rep", [D], FP, kind="ExternalInput")
    w_head_d = nc.dram_tensor("w_head", [D, D], FP, kind="ExternalInput")
    w_dep_d = nc.dram_tensor("w_dep", [D, D], FP, kind="ExternalInput")
    b_attn_d = nc.dram_tensor("b_attn", [D], FP, kind="ExternalInput")
    w_frep_d = nc.dram_tensor("w_frep", [D, D], FP, kind="ExternalInput")
    w_fattn_d = nc.dram_tensor("w_fattn", [D, D], FP, kind="ExternalInput")
    b_f_d = nc.dram_tensor("b_f", [D], FP, kind="ExternalInput")
    out_d = nc.dram_tensor("out", [BC, S, D], FP, kind="ExternalOutput")

    with tile.TileContext(nc) as tc:
        _emit(nc, tc, reps, x_d, w_rep_d, b_rep_d, w_head_d, w_dep_d,
              b_attn_d, w_frep_d, w_fattn_d, b_f_d, out_d)
    nc.compile()
    return nc


def _emit(nc, tc, reps, x_d, w_rep_d, b_rep_d, w_head_d, w_dep_d,
          b_attn_d, w_frep_d, w_fattn_d, b_f_d, out_d):
    from contextlib import ExitStack
    ctx = ExitStack()
    with ctx:
        const = ctx.enter_context(tc.tile_pool(name="const", bufs=1))
        wpool = ctx.enter_context(tc.tile_pool(name="weights", bufs=1))
        small = ctx.enter_context(tc.tile_pool(name="small", bufs=4))
        keep = ctx.enter_context(tc.tile_pool(name="keep", bufs=1))
        tpool = ctx.enter_context(tc.tile_pool(name="tbuf", bufs=6))
        accp = ctx.enter_context(tc.tile_pool(name="accum", bufs=1))
        epool = ctx.enter_context(tc.tile_pool(name="ebuf", bufs=6))
        psum = ctx.enter_context(tc.tile_pool(name="psum", bufs=3,
                                              space="PSUM"))
        psmall = ctx.enter_context(tc.tile_pool(name="psmall", bufs=2,
                                                space="PSUM"))

        ident = const.tile([128, 128], FP, tag="ident")
        make_identity(nc, ident[:])
        ident_r = const.tile([128, 128], FP, tag="ident_r")
        nc.scalar.copy(out=_rr(ident_r[:]), in_=ident[:])

        # --- replicated weights: W[dh][eh] = W[128dh:.., 128eh:..] ---
        def load_w(dram, nm):
            tiles = []
            for dh in range(NH):
                row = []
                for eh in range(NH):
                    t0 = small.tile([128, 128], FP, tag="wload")
                    nc.sync.dma_start(
                        out=t0[:],
                        in_=dram.ap()[128 * dh:128 * (dh + 1),
                                      128 * eh:128 * (eh + 1)])
                    t = wpool.tile([128, 128], FP, tag=f"{nm}_{dh}_{eh}")
                    nc.scalar.copy(out=t[:], in_=t0[:])
                    row.append(t)
                tiles.append(row)
            return tiles

        w_rep = load_w(w_rep_d, "wrep")
        w_dep = load_w(w_dep_d, "wdep")
        w_head = load_w(w_head_d, "whead")
        w_frep = load_w(w_frep_d, "wfrep")
        w_fattn = load_w(w_fattn_d, "wfattn")

        # bias columns as [128, NH] tiles (column h = bias[128h : 128h+128])
        def load_b(dram, tag):
            t = wpool.tile([128, NH], FP, tag=tag)
            nc.sync.dma_start(out=t[:],
                              in_=dram.ap().rearrange("(h p) -> p h", p=128))
            return t

        b_rep_c = load_b(b_rep_d, "b_rep")
        b_attn_c = load_b(b_attn_d, "b_attn")
        b_f_c = load_b(b_f_d, "b_f")
        b_attn_s = wpool.tile([128, NH], FP, tag="b_attn_s")   # b_attn/SCALE
        nc.vector.tensor_scalar_mul(b_attn_s[:], b_attn_c[:], 1.0 / SCALE)
        b_f_n = wpool.tile([128, NH], FP, tag="b_f_n")         # -b_f
        nc.vector.tensor_scalar_mul(b_f_n[:], b_f_c[:], -1.0)

        def body(_iv=None):
            # R[b][h]   : rep_map^T [e-half, s]   (lhsT/rhs + E-multiply src)
            # depp[b][h]: dep  natural [s, d-half] (stationary for U dep-part)
            # headp[b][h]: head natural [s, d-half]
            R = [[None] * NH for _ in range(BC)]
            depp = [[None] * NH for _ in range(BC)]
            headp = [[None] * NH for _ in range(BC)]
            attnT = [[None] * NH for _ in range(BC)]

            for b in range(BC):
                # ---------- phase A: small matmuls ----------
                xt = []
                for h in range(NH):
                    xs = small.tile([128, 128], FP, tag="x_in")
                    nc.sync.dma_start(
                        out=xs[:], in_=x_d.ap()[b, :, 128 * h:128 * (h + 1)])
                    pt = psmall.tile([128, 512], FP, tag="pss")
                    nc.tensor.transpose(pt[:, :128], xs[:], ident[:])
                    xth = small.tile([128, 128], FP, tag="xt")
                    nc.scalar.copy(out=_r(xth[:]), in_=pt[:, :128])
                    xt.append(xth)

                for eh in range(NH):
                    # rep_map^T[e,s] = sum_d W_rep[d,e] * x^T[d,s]
                    pre = psmall.tile([128, 512], FP, tag="pss")
                    nc.tensor.matmul(out=pre[:, :128], lhsT=_r(w_rep[0][eh][:]),
                                     rhs=_r(xt[0][:]), start=True, stop=False)
                    nc.tensor.matmul(out=pre[:, :128], lhsT=_r(w_rep[1][eh][:]),
                                     rhs=_r(xt[1][:]), start=False, stop=True)
                    # elu(z) = relu(z) + exp(min(z, 0)) - 1,  z = pre + b_rep
                    rpos = small.tile([128, 128], FP, tag="rpos")
                    nc.scalar.activation(out=rpos[:], in_=pre[:, :128],
                                         func=AF.Relu,
                                         bias=b_rep_c[:, eh:eh + 1])
                    zneg = small.tile([128, 128], FP, tag="zneg")
                    nc.vector.tensor_scalar(out=zneg[:], in0=pre[:, :128],
                                            scalar1=b_rep_c[:, eh:eh + 1],
                                            scalar2=0.0, op0=ALU.add,
                                            op1=ALU.min)
                    ez = small.tile([128, 128], FP, tag="ez")
                    nc.scalar.activation(out=ez[:], in_=zneg[:], func=AF.Exp)
                    Rt = keep.tile([128, 128], FP, tag=f"R_{b}_{eh}")
                    nc.vector.scalar_tensor_tensor(
                        out=_r(Rt[:]), in0=ez[:], scalar=-1.0, in1=rpos[:],
                        op0=ALU.add, op1=ALU.add)
                    R[b][eh] = Rt

                for eh in range(NH):
                    # dep[s,e'] = sum_e rep_map[s,e] W_dep[e,e']
                    #           = (rep_map^T as lhsT).T @ W_dep
                    pd = psmall.tile([128, 512], FP, tag="pss")
                    nc.tensor.matmul(out=pd[:, :128], lhsT=_r(R[b][0][:]),
                                     rhs=_r(w_dep[0][eh][:]), start=True,
                                     stop=False)
                    nc.tensor.matmul(out=pd[:, :128], lhsT=_r(R[b][1][:]),
                                     rhs=_r(w_dep[1][eh][:]), start=False,
                                     stop=True)
                    # compensated f32r: dep = hi + lo, each f32r-rounded
                    dh_ = keep.tile([128, 128], FP, tag=f"dep_h_{b}_{eh}",
                                    name=f"dep_h_{b}_{eh}")
                    nc.scalar.copy(out=_rr(dh_[:]), in_=pd[:, :128])
                    dl_ = keep.tile([128, 128], FP, tag=f"dep_l_{b}_{eh}",
                                    name=f"dep_l_{b}_{eh}")
                    nc.vector.tensor_sub(_rr(dl_[:]), pd[:, :128], dh_[:])
                    depp[b][eh] = (dh_, dl_)

                    ph = psmall.tile([128, 512], FP, tag="pss")
                    nc.tensor.matmul(out=ph[:, :128], lhsT=_r(R[b][0][:]),
                                     rhs=_r(w_head[0][eh][:]), start=True,
                                     stop=False)
                    nc.tensor.matmul(out=ph[:, :128], lhsT=_r(R[b][1][:]),
                                     rhs=_r(w_head[1][eh][:]), start=False,
                                     stop=True)
                    hh_ = keep.tile([128, 128], FP, tag=f"head_h_{b}_{eh}",
                                    name=f"head_h_{b}_{eh}")
                    nc.scalar.copy(out=_rr(hh_[:]), in_=ph[:, :128])
                    hl_ = keep.tile([128, 128], FP, tag=f"head_l_{b}_{eh}",
                                    name=f"head_l_{b}_{eh}")
                    nc.vector.tensor_sub(_rr(hl_[:]), ph[:, :128], hh_[:])
                    headp[b][eh] = (hh_, hl_)

            def phase_c(b):
                for eh in range(NH):
                    pg = psmall.tile([128, 512], FP, tag="pss")
                    nc.tensor.matmul(out=pg[:, :128], lhsT=_r(w_fattn[0][eh][:]),
                                     rhs=_r(attnT[b][0][:]), start=True,
                                     stop=False)
                    nc.tensor.matmul(out=pg[:, :128], lhsT=_r(w_fattn[1][eh][:]),
                                     rhs=_r(attnT[b][1][:]), start=False,
                                     stop=False)
                    nc.tensor.matmul(out=pg[:, :128], lhsT=_r(w_frep[0][eh][:]),
                                     rhs=_r(R[b][0][:]), start=False, stop=False)
                    nc.tensor.matmul(out=pg[:, :128], lhsT=_r(w_frep[1][eh][:]),
                                     rhs=_r(R[b][1][:]), start=False, stop=True)
                    # sigmoid(z) = 1/(1 + exp(-z)); stays in the exp table set
                    eg = small.tile([128, 128], FP, tag="eg")
                    nc.scalar.activation(out=eg[:], in_=pg[:, :128],
                                         func=AF.Exp, scale=-1.0,
                                         bias=b_f_n[:, eh:eh + 1])
                    nc.vector.tensor_scalar_add(eg[:], eg[:], 1.0)
                    gate = small.tile([128, 128], FP, tag="gate")
                    nc.vector.reciprocal(out=gate[:], in_=eg[:])
                    # out^T = attn + gate * (rep - attn)
                    diff = small.tile([128, 128], FP, tag="diff")
                    nc.vector.tensor_sub(diff[:], R[b][eh][:],
                                         attnT[b][eh][:])
                    nc.vector.tensor_mul(diff[:], gate[:], diff[:])
                    nc.vector.tensor_add(diff[:], diff[:], attnT[b][eh][:])
                    # transpose [e, s] -> [s, e] and store
                    po = psmall.tile([128, 512], FP, tag="pss")
                    nc.tensor.transpose(po[:, :128], diff[:], ident[:])
                    osb = small.tile([128, 128], FP, tag="osb")
                    nc.scalar.copy(out=osb[:], in_=po[:, :128])
                    nc.sync.dma_start(
                        out=out_d.ap()[b, :, 128 * eh:128 * (eh + 1)],
                        in_=osb[:])


            # ---------- phase B: the S^2 x D attention core ----------
            # bi-blocks interleaved across the 4 (b, h) units so four
            # independent dependency chains are always in flight.
            units = [(b, h) for b in range(BC) for h in range(NH)]
            nums = {}
            dens = {}
            for u, (b, h) in enumerate(units):
                nums[u] = accp.tile([128, 128], FP, tag=f"num_{u}", name=f"num_{u}")
                dens[u] = accp.tile([128, 128], FP, tag=f"den_{u}", name=f"den_{u}")
            for bi in range(NBI):
                L = BLK * (bi + 1)
                Tbs, Ebs = {}, {}
                # stage 1: tensor engine U chunks + tanh (ACT stays dense)
                for u, (b, h) in enumerate(units):
                    Tb = tpool.tile([128, BLK, 128], FP, tag="T")
                    Tbs[u] = Tb
                    i_done = 0
                    for chunk in _BLOCK_PLAN[bi]:
                        pu = psum.tile([128, 1024], FP, tag="ps")
                        n_i = chunk[0][1]
                        for pi, (i0, n_i_p) in enumerate(chunk):
                            cols = n_i_p * L
                            sl = pu[:, 512 * pi:512 * pi + cols]
                            # dep part: col (im, j) reads ident col j
                            selj = _bcast(ident_r[:], [[0, n_i_p], [1, L]])
                            # head part: col (im, j) reads ident col
                            # (BLK*bi + i0 + im), repeated L times
                            seli = _bcast(ident_r[:], [[1, n_i_p], [0, L]],
                                          extra_offset=BLK * bi + i0)
                            dh_, dl_ = depp[b][h]
                            hh_, hl_ = headp[b][h]
                            for k, (w, sel) in enumerate(
                                    [(dh_, selj), (hh_, seli),
                                     (dl_, selj), (hl_, seli)]):
                                nc.tensor.matmul(out=sl, lhsT=_rr(w[:]),
                                                 rhs=_rr(sel),
                                                 start=(k == 0),
                                                 stop=(k == 3))
                        # tanh((U + b_attn)/SCALE): PSUM -> packed T rows
                        src_ = _bcast(pu[:], [[512, len(chunk)],
                                              [L, n_i], [1, L]])
                        dst = _bcast(Tb[:], [[128 * n_i, len(chunk)],
                                             [128, n_i], [1, L]],
                                     extra_offset=128 * i_done)
                        nc.scalar.activation(out=dst, in_=src_,
                                             func=AF.Tanh,
                                             bias=b_attn_s[:, h:h + 1],
                                             scale=1.0 / SCALE)
                        i_done += sum(n for _, n in chunk)
                # stage 2: E = exp(SCALE * T)
                for u, (b, h) in enumerate(units):
                    Eb = epool.tile([128, BLK, 128], FP, tag="E")
                    Ebs[u] = Eb
                    tview = _bcast(Tbs[u][:], [[128, BLK], [1, L]])
                    eview = _bcast(Eb[:], [[128, BLK], [1, L]])
                    nc.scalar.activation(out=eview, in_=tview,
                                         func=AF.Exp, scale=SCALE)
                # stage 3: gpsimd causal mask on E's diagonal BLK j-columns
                # keep where (im - jm - 1) >= 0  <=>  j < i ; else E := 0
                for u, (b, h) in enumerate(units):
                    diag = _bcast(Ebs[u][:], [[128, BLK], [1, BLK]],
                                  extra_offset=L - BLK)
                    nc.gpsimd.affine_select(
                        out=diag, in_=diag, compare_op=ALU.is_ge,
                        fill=0.0, base=-1, channel_multiplier=0,
                        pattern=[[1, BLK], [-1, BLK]])
                # stage 4: segmented den reductions (need only E)
                for u, (b, h) in enumerate(units):
                    nc.vector.tensor_reduce(
                        out=dens[u][:, BLK * bi:BLK * (bi + 1)],
                        in_=_bcast(Ebs[u][:], [[128, BLK], [1, L]]),
                        axis=mybir.AxisListType.X, op=ALU.add)
                # stage 5: M = E * rep (rep bcast over i); reuse T as M
                for u, (b, h) in enumerate(units):
                    rrep = _bcast(R[b][h][:], [[0, BLK], [1, L]])
                    tview = _bcast(Tbs[u][:], [[128, BLK], [1, L]])
                    eview = _bcast(Ebs[u][:], [[128, BLK], [1, L]])
                    eng = nc.vector if bi <= 1 else nc.gpsimd
                    eng.tensor_tensor(out=tview, in0=eview, in1=rrep,
                                      op=ALU.mult)
                # stage 6: segmented num reductions
                for u, (b, h) in enumerate(units):
                    nc.vector.tensor_reduce(
                        out=nums[u][:, BLK * bi:BLK * (bi + 1)],
                        in_=_bcast(Tbs[u][:], [[128, BLK], [1, L]]),
                        axis=mybir.AxisListType.X, op=ALU.add)
            for u, (b, h) in enumerate(units):
                num, den = nums[u], dens[u]
                # attn^T = num / (den + eps)
                nc.vector.tensor_scalar_add(den[:], den[:], DEN_EPS)
                nc.vector.reciprocal(out=den[:], in_=den[:])
                at = keep.tile([128, 128], FP, tag=f"attn_{b}_{h}")
                nc.vector.tensor_tensor(out=_r(at[:]), in0=num[:],
                                        in1=den[:], op=ALU.mult)
                attnT[b][h] = at
                if h == NH - 1:
                    phase_c(b)
        if reps == 1:
            body()
        else:
            with tc.For_i(0, reps, 1) as iv:
                body(iv)


_CACHED = {}


def _get_program(reps=1):
    if reps not in _CACHED:
        _CACHED[reps] = build_program(reps)
    return _CACHED[reps]


def make_in_maps(inputs):
    x = np.ascontiguousarray(np.asarray(inputs["x"], dtype=np.float32))
    names = {
        "w_rep": inputs["rep_map_kernel"], "b_rep": inputs["rep_map_bias"],
        "w_head": inputs["head_kernel"], "w_dep": inputs["dependent_kernel"],
        "b_attn": inputs["attn_bias"], "w_frep": inputs["f_rep_kernel"],
        "w_fattn": inputs["f_attn_kernel"], "b_f": inputs["f_bias"],
    }
    shared = {k: np.ascontiguousarray(np.asarray(v, dtype=np.float32))
              for k, v in names.items()}
    return [dict(shared, x=x[c * BC:(c + 1) * BC]) for c in range(NCORES)]


def kernel(**inputs):
    nc = _get_program(reps=1)
    in_maps = make_in_maps(inputs)
    res = run_bass_kernel_spmd(nc, in_maps, list(range(NCORES)))
    out = np.concatenate([res.results[c]["out"] for c in range(NCORES)],
                         axis=0)
    return out.astype(np.float32)

